# revision 27
# baseline (speedup 1.0000x reference)
"""Trainium2 Bass kernel for a dense transformer block (B=4, N=2048, C=768,
H=12, D=64, HID=3072), sharded over 8 NeuronCores.

Sharding: token-split, no collectives. Core s handles batch b = s//2,
sequence half = s%2 (1024 tokens). Each core receives its batch element's
full 2048-token x (rolled so its own tokens are rows 0..1023), computes
K/V over all 2048 tokens (redundantly with its pair core), and produces
the output for its own 1024 tokens. Host gathers/transposes.

v2 schedule: attention is ScalarE(exp)-bound, so the PE instruction
stream is interleaved with independent matmul work ("fillers"): K/Q
feature tiles are computed between chunk-0 head-pairs, and proj/LN2/fc1
(chunk 0) run between chunk-1 head-pairs. S->exp->AV runs software-
pipelined with a 2-step skew so the PE never waits long on the Scalar
engine. Softmax denominators are drained to SBUF ([12,512] per chunk)
and inverted with ONE batched DVE reciprocal per chunk (free-dim-bound
op). All rsqrt/reciprocal-style scalars use Ln+Exp (one activation
table set shared with the softmax exp; Gelu is the only other set).
Residual streams xoT/x2T are bf16 to fit SBUF; PSUM pools are 3+3+2
banks (S tiles / AV accumulators / proj-LN2-MLP).
"""

from contextlib import ExitStack

import numpy as np

import concourse.bass as bass
import concourse.mybir as mybir
import concourse.tile as tile
from concourse import bacc
from concourse.bass_utils import run_bass_kernel_spmd
from concourse.masks import make_identity

F32 = mybir.dt.float32
F32R = mybir.dt.float32r
BF16 = mybir.dt.bfloat16
AF = mybir.ActivationFunctionType
ALU = mybir.AluOpType

B, N, C = 4, 2048, 768
H, D = 12, 64
HID = 3072
EPS = 1e-5
NCORES = 8
NO = 1024  # tokens owned per core
NKV = 2048  # key/value tokens per core
CT = C // 128  # 6 feature tiles
HT = HID // 128  # 24 hidden tiles
HQ = HT // 2  # 12 hidden tiles per half
KT = NKV // 128  # 16 kv token tiles
QCH = NO // 512  # 2 query chunks of 512
ISCALE = 1.0 / np.sqrt(D)
SKEW = 3  # S->AV software-pipeline depth (in nt steps)

LAST_RESULTS = None
_NC_CACHE = None


def build_program(repeats=1):
    nc = bacc.Bacc(trn_type="TRN2", target_bir_lowering=False, num_devices=NCORES)

    xb = nc.dram_tensor("xb", [NKV, C], F32, kind="ExternalInput").ap()
    wqkvT = nc.dram_tensor("wqkvT", [C, 3 * C], BF16, kind="ExternalInput").ap()
    wprojT = nc.dram_tensor("wprojT", [C, C], BF16, kind="ExternalInput").ap()
    wfc1T = nc.dram_tensor("wfc1T", [C, HID], BF16, kind="ExternalInput").ap()
    wfc2T = nc.dram_tensor("wfc2T", [HID, C], BF16, kind="ExternalInput").ap()
    pb = nc.dram_tensor("pb", [C], F32, kind="ExternalInput").ap()
    f1b = nc.dram_tensor("f1b", [HID], F32, kind="ExternalInput").ap()
    f2b = nc.dram_tensor("f2b", [C], F32, kind="ExternalInput").ap()
    g1 = nc.dram_tensor("g1", [C], F32, kind="ExternalInput").ap()
    b1 = nc.dram_tensor("b1", [C], F32, kind="ExternalInput").ap()
    g2 = nc.dram_tensor("g2", [C], F32, kind="ExternalInput").ap()
    b2 = nc.dram_tensor("b2", [C], F32, kind="ExternalInput").ap()
    outT = nc.dram_tensor("outT", [C, NO], F32, kind="ExternalOutput").ap()

    with tile.TileContext(nc) as tc:
        for _ in range(repeats):
            emit(nc, tc, xb, wqkvT, wprojT, wfc1T, wfc2T, pb, f1b, f2b,
                 g1, b1, g2, b2, outT)
    nc.compile()
    return nc


def emit(nc, tc, xb, wqkvT, wprojT, wfc1T, wfc2T, pb, f1b, f2b,
         g1, b1, g2, b2, outT):
    dma = nc.sync.dma_start
    qs = (slice(0, 64), slice(64, 128))
    qs2 = qs

    with ExitStack() as _st:
        def pool(**kw):
            return _st.enter_context(tc.tile_pool(**kw))

        consts = pool(name="consts", bufs=1)
        x2T_pool = pool(name="x2T", bufs=1)
        xoT_pool = pool(name="xoT", bufs=1)
        # ---- constants ----
        ident = consts.tile([128, 128], F32, tag="ident")
        make_identity(nc, ident)
        ident_bf = consts.tile([128, 128], BF16, tag="ident_bf")
        make_identity(nc, ident_bf)
        ones_f32 = consts.tile([128, 1], F32, tag="ones_f32")
        nc.vector.memset(ones_f32, 1.0)
        ones_cb = consts.tile([128, 1], BF16, tag="ones_cb")
        nc.scalar.activation(out=ones_cb, in_=ones_f32, func=AF.Copy)
        eps_t = consts.tile([128, 1], F32, tag="eps")
        nc.vector.memset(eps_t, EPS)
        g1_s = consts.tile([128, CT], F32, tag="g1")
        dma(out=g1_s, in_=g1.rearrange("(ct p) -> p ct", p=128))
        b1_s = consts.tile([128, CT], F32, tag="b1")
        dma(out=b1_s, in_=b1.rearrange("(ct p) -> p ct", p=128))
        g2_s = consts.tile([128, CT], F32, tag="g2")
        dma(out=g2_s, in_=g2.rearrange("(ct p) -> p ct", p=128))
        b2_s = consts.tile([128, CT], F32, tag="b2")
        dma(out=b2_s, in_=b2.rearrange("(ct p) -> p ct", p=128))
        pb_s = consts.tile([128, CT], F32, tag="pb")
        dma(out=pb_s, in_=pb.rearrange("(ct p) -> p ct", p=128))
        f2b_s = consts.tile([128, CT], F32, tag="f2b")
        dma(out=f2b_s, in_=f2b.rearrange("(ct p) -> p ct", p=128))
        f1b_s = consts.tile([128, HT], F32, tag="f1b")
        dma(out=f1b_s, in_=f1b.rearrange("(ht p) -> p ht", p=128))

        # residual streams (bf16 to fit SBUF; rel-err budget is 2e-2)
        x2T = [x2T_pool.tile([128, NO], BF16, tag=f"x2T{ct}", name=f"x2T{ct}")
               for ct in range(CT)]
        xoT = [xoT_pool.tile([128, NO], BF16, tag=f"xoT{ct}", name=f"xoT{ct}")
               for ct in range(CT)]

        if True:
            oU_pool = pool(name="oU", bufs=12)
            oT_pool = pool(name="oT", bufs=6)
            den_pool = pool(name="den", bufs=1)
            asm = pool(name="asm", bufs=2)
            sps = pool(name="s_psum", bufs=2, space="PSUM")
            ops = pool(name="o_psum", bufs=2, space="PSUM")
            mps = pool(name="m_psum", bufs=2, space="PSUM")

            _stk = ExitStack()  # last-opened left pools: freed after attention

            def kpool(**kw):
                return _stk.enter_context(tc.tile_pool(**kw))

            qT_pool = kpool(name="qT", bufs=1)
            kT_pool = kpool(name="kT", bufs=1)
            vA_pool = kpool(name="vA", bufs=1)
            p_sb = kpool(name="p_sb", bufs=6)
            qT = [qT_pool.tile([128, NO], BF16, tag=f"qT{ct}", name=f"qT{ct}")
                  for ct in range(CT)]
            kT = [kT_pool.tile([128, NKV], BF16, tag=f"kT{ct}", name=f"kT{ct}")
                  for ct in range(CT)]
            vA = [vA_pool.tile([128, H, D + 1], BF16, tag=f"vA{nt}",
                               name=f"vA{nt}") for nt in range(KT)]
            dent = [None, None]
            recT = [None, None]
            # oT[ch][hp]: normalized attention output, feature-major
            oT = [[None] * CT for _ in range(QCH)]
            ln2v = [None, None]

            # ---------------- attention inner machinery ----------------
            def attn_hp(ch, hp):
                """S->exp->AV pipeline for head pair hp, query chunk ch."""
                qch = slice(ch * 512, (ch + 1) * 512)
                po = [ops.tile([D + 1, 512], F32, tag="po",
                               name=f"po{ch}_{hp}_{i}") for i in range(2)]
                pts = {}
                for step in range(KT + SKEW):
                    if step < KT:
                        nt = step
                        ps2 = sps.tile([128, 1024], F32, tag="ps")
                        for i in range(2):
                            nc.tensor.matmul(
                                ps2[:, i * 512:(i + 1) * 512],
                                kT[hp][qs[i], nt * 128:(nt + 1) * 128],
                                qT[hp][qs[i], qch],
                                start=True, stop=True,
                                tile_position=(64 * i, 0))
                        pt2 = p_sb.tile([128, 1024], BF16, tag="pt")
                        nc.scalar.activation(out=pt2, in_=ps2,
                                             func=AF.Exp, scale=ISCALE)
                        pts[nt] = pt2
                    if step >= SKEW:
                        m = step - SKEW
                        pt2 = pts.pop(m)
                        for i in range(2):
                            nc.tensor.matmul(
                                po[i], vA[m][:, 2 * hp + i, :],
                                pt2[:, i * 512:(i + 1) * 512],
                                start=(m == 0), stop=(m == KT - 1),
                                skip_group_check=True)
                # drain: unnormalized O^T halves + denominator row
                oUs = []
                for i in range(2):
                    r = 2 * hp + i
                    oU = oU_pool.tile([D, 512], BF16, tag="oU",
                                      name=f"oU{ch}_{r}")
                    nc.vector.tensor_copy(out=oU, in_=po[i][0:D, :])
                    oUs.append(oU)
                    dt = asm.tile([1, 512], BF16, tag="dtmp")
                    nc.vector.tensor_copy(out=dt, in_=po[i][D:D + 1, :])
                    dma(out=dent[ch][r:r + 1, :], in_=dt)
                return oUs

            oU_all = [[None] * H for _ in range(QCH)]

            def attn_chunk(ch, fillers):
                dent[ch] = den_pool.tile([H, 512], BF16, tag="dent",
                                         name=f"dent{ch}")
                for hp in range(CT):
                    oUs = attn_hp(ch, hp)
                    oU_all[ch][2 * hp] = oUs[0]
                    oU_all[ch][2 * hp + 1] = oUs[1]
                    if hp < len(fillers) and fillers[hp] is not None:
                        fillers[hp]()

            def ot_norm(ch):
                """One batched reciprocal, then per-head broadcast+mul."""
                recT[ch] = den_pool.tile([H, 512], BF16, tag="recT",
                                         name=f"recT{ch}")
                with nc.allow_low_precision(reason="softmax denom in bf16"):
                    nc.vector.reciprocal(out=recT[ch], in_=dent[ch])
                for hp in range(CT):
                    t = oT_pool.tile([128, 512], BF16, tag="oT",
                                     name=f"oT{ch}_{hp}")
                    for i in range(2):
                        r = 2 * hp + i
                        rb = asm.tile([1, 512], BF16, tag="rb")
                        dma(out=rb, in_=recT[ch][r:r + 1, :])
                        vb = asm.tile([D, 512], BF16, tag="vb")
                        nc.gpsimd.partition_broadcast(vb, rb, channels=D)
                        nc.vector.tensor_mul(t[qs2[i], :], oU_all[ch][r], vb)
                    oT[ch][hp] = t

            # ------------- proj / LN2 / MLP building blocks -------------
            def proj_piece(ch, fts, wp):
                cs = slice(ch * 512, (ch + 1) * 512)
                for ft in fts:
                    ps = mps.tile([128, 512], F32, tag="mp", name="ppj")
                    for ct in range(CT):
                        nc.tensor.matmul(
                            ps, wp[:, ct, ft * 128:(ft + 1) * 128],
                            oT[ch][ct],
                            start=(ct == 0), stop=(ct == CT - 1))
                    nc.vector.scalar_tensor_tensor(
                        out=x2T[ft][:, cs],
                        in0=ps, scalar=pb_s[:, ft:ft + 1],
                        in1=xoT[ft][:, cs],
                        op0=ALU.add, op1=ALU.add)

            def ln2_chunk(ch, lw):
                cs = slice(ch * 512, (ch + 1) * 512)
                psum = mps.tile([1, 512], F32, tag="mp", name="psum")
                pssq = mps.tile([1, 512], F32, tag="mp", name="pssq")
                for ct in range(CT):
                    sq = lw.tile([128, 512], BF16, tag="sq", bufs=1)
                    nc.vector.tensor_mul(sq, x2T[ct][:, cs], x2T[ct][:, cs])
                    nc.tensor.matmul(psum, ones_cb, x2T[ct][:, cs],
                                     start=(ct == 0), stop=(ct == CT - 1),
                                     skip_group_check=True)
                    nc.tensor.matmul(pssq, ones_cb, sq,
                                     start=(ct == 0), stop=(ct == CT - 1),
                                     skip_group_check=True)
                mu = lw.tile([1, 512], F32, tag="mu", name=f"mu{ch}", bufs=1)
                nc.scalar.mul(mu, psum, 1.0 / C)
                msq = lw.tile([1, 512], F32, tag="msq", bufs=1)
                nc.scalar.mul(msq, pssq, 1.0 / C)
                mu2 = lw.tile([1, 512], F32, tag="mu2", bufs=1)
                nc.vector.tensor_mul(mu2, mu, mu)
                nc.vector.tensor_sub(msq, msq, mu2)  # msq := var
                nc.scalar.activation(out=mu2, in_=msq, func=AF.Sqrt,
                                     bias=eps_t[0:1], scale=1.0)
                rstdf = lw.tile([1, 512], F32, tag="rstdf", bufs=1)
                nc.vector.reciprocal(out=rstdf, in_=mu2)
                rstd = lw.tile([1, 512], BF16, tag=f"rstd2_{ch}",
                               name="rstd2", bufs=1)
                nc.vector.tensor_copy(out=rstd, in_=rstdf)
                nmr = lw.tile([1, 512], BF16, tag=f"nmr2_{ch}",
                              name="nmr2", bufs=1)
                nc.vector.scalar_tensor_tensor(
                    out=nmr, in0=mu, scalar=-1.0, in1=rstdf,
                    op0=ALU.mult, op1=ALU.mult)
                ln2v[ch] = (rstd, nmr)

            h2c = [[None] * CT for _ in range(QCH)]

            def h2_prep(ch, lw, h2_pool, bc_pool):
                cs = slice(ch * 512, (ch + 1) * 512)
                rstd, nmr = ln2v[ch]
                bc_r = bc_pool.tile([128, 512], BF16, tag="bc_r")
                nc.gpsimd.partition_broadcast(bc_r, rstd, channels=128)
                bc_m = bc_pool.tile([128, 512], BF16, tag="bc_m")
                nc.gpsimd.partition_broadcast(bc_m, nmr, channels=128)
                for ct in range(CT):
                    t = lw.tile([128, 512], BF16, tag="h2tmp")
                    nc.vector.tensor_mul(t, x2T[ct][:, cs], bc_r)
                    nc.vector.tensor_add(t, t, bc_m)
                    h2 = h2_pool.tile([128, 512], BF16,
                                      tag=f"h2_{ct}_{ch}", name=f"h2_{ct}_{ch}")
                    nc.scalar.activation(
                        out=h2, in_=t, func=AF.Identity,
                        scale=g2_s[:, ct:ct + 1], bias=b2_s[:, ct:ct + 1])
                    h2c[ch][ct] = h2

            # fc1 matmuls only (stage pre-gelu h1 in bf16; gelu deferred so
            # the Scalar engine stays on the exp table set during attention)
            def fc1_piece(ch, hb, hts, w1t, gbuf, h1s):
                for ht in hts:
                    ps = mps.tile([128, 512], F32, tag="mp", name="f1")
                    for ct in range(CT):
                        nc.tensor.matmul(
                            ps, w1t[:, ct, ht * 128:(ht + 1) * 128],
                            h2c[ch][ct],
                            start=(ct == 0), stop=(ct == CT - 1))
                    h1 = gbuf.tile([128, 512], BF16, tag="gb",
                                   name=f"h1_{ch}_{hb}_{ht}")
                    nc.vector.tensor_copy(out=h1, in_=ps)
                    h1s[ht] = h1

            def gelu_piece(ch, hb, hts, gbuf, h1s, gs):
                for ht in hts:
                    g = gbuf.tile([128, 512], BF16, tag="gb",
                                  name=f"g_{ch}_{hb}_{ht}")
                    hti = hb * HQ + ht
                    nc.scalar.activation(out=g, in_=h1s[ht], func=AF.Gelu,
                                         bias=f1b_s[:, hti:hti + 1], scale=1.0)
                    gs[ht] = g

            # fc1 with gelu fused right after (for post-attention phases)
            def fc1_full(ch, hb, w1t, gbuf, gs):
                for ht in range(HQ):
                    ps = mps.tile([128, 512], F32, tag="mp", name="f1")
                    for ct in range(CT):
                        nc.tensor.matmul(
                            ps, w1t[:, ct, ht * 128:(ht + 1) * 128],
                            h2c[ch][ct],
                            start=(ct == 0), stop=(ct == CT - 1))
                    g = gbuf.tile([128, 512], BF16, tag="gb",
                                  name=f"g_{ch}_{hb}_{ht}")
                    hti = hb * HQ + ht
                    nc.scalar.activation(out=g, in_=ps, func=AF.Gelu,
                                         bias=f1b_s[:, hti:hti + 1], scale=1.0)
                    gs[ht] = g

            acc = {}

            def fc2_piece(ch, hb, fts, w2t, gs, accp, osb):
                cs = slice(ch * 512, (ch + 1) * 512)
                for ft in fts:
                    ps = mps.tile([128, 512], F32, tag="mp", name="f2")
                    for ht in range(HQ):
                        nc.tensor.matmul(
                            ps, w2t[:, ht, ft * 128:(ft + 1) * 128], gs[ht],
                            start=(ht == 0), stop=(ht == HQ - 1))
                    if hb == 0:
                        a = accp.tile([128, 512], BF16, tag=f"acc{ft}_{ch}",
                                      name=f"acc{ft}_{ch}")
                        acc[(ft, ch)] = a
                        nc.vector.scalar_tensor_tensor(
                            out=a, in0=ps,
                            scalar=f2b_s[:, ft:ft + 1],
                            in1=x2T[ft][:, cs],
                            op0=ALU.add, op1=ALU.add)
                    else:
                        ot = osb.tile([128, 512], F32, tag="ot")
                        nc.vector.tensor_add(ot, ps, acc[(ft, ch)])
                        dma(out=outT[ft * 128:(ft + 1) * 128, cs], in_=ot)

            # ============ Phase 1+2: LN1, transposes, QKV, attn(ch0) ======
            with (
                tc.tile_pool(name="hkvT", bufs=1) as hkvT_pool,
                tc.tile_pool(name="wqkv", bufs=1) as wq_pool,
            ):
                hkvT = [hkvT_pool.tile([128, NKV], BF16, tag=f"hkvT{ct}",
                                       name=f"hkvT{ct}") for ct in range(CT)]
                wq = wq_pool.tile([128, CT, 3 * C], BF16, tag="wqkv")
                with (
                    tc.tile_pool(name="ln1_work", bufs=2) as lw1,
                    tc.tile_pool(name="ln1_stat", bufs=6) as lstat,
                ):
                    for g in range(KT // 4):  # groups of 4 token tiles
                        xts, xcs = [], []
                        for j in range(4):
                            nt = 4 * g + j
                            xt = lw1.tile([128, C], F32, tag=f"xt{j}",
                                          name=f"xt{j}")
                            dma(out=xt, in_=xb[nt * 128:(nt + 1) * 128, :])
                            st = lstat.tile([128, 3, 6], F32, tag="st")
                            xg = xt.rearrange("p (s d) -> p s d", s=3)
                            for s in range(3):
                                nc.vector.bn_stats(out=st[:, s], in_=xg[:, s])
                            mv = lstat.tile([128, 2], F32, tag="mv")
                            nc.vector.bn_aggr(out=mv, in_=st)
                            rstd = lstat.tile([128, 1], F32, tag="rstd")
                            nc.scalar.activation(out=rstd, in_=mv[:, 1:2],
                                                 func=AF.Sqrt,
                                                 bias=eps_t, scale=1.0)
                            nc.vector.reciprocal(out=rstd, in_=rstd)
                            nmr = lstat.tile([128, 1], F32, tag="nmr")
                            nc.vector.tensor_scalar(out=nmr, in0=mv[:, 0:1],
                                                    scalar1=-1.0, scalar2=rstd,
                                                    op0=ALU.mult, op1=ALU.mult)
                            xc = lw1.tile([128, C], BF16, tag=f"xc{j}",
                                          name=f"xc{j}", bufs=1)
                            nc.scalar.activation(out=xc, in_=xt,
                                                 func=AF.Identity,
                                                 scale=rstd, bias=nmr)
                            xts.append(xt)
                            xcs.append(xc)
                        if g == 0:
                            dma(out=wq,
                                in_=wqkvT.rearrange("(ct p) f -> p ct f", p=128))
                        for ct in range(CT):
                            ps = sps.tile([128, 512], BF16, tag="ps")
                            for j in range(4):
                                nc.tensor.transpose(
                                    ps[:, j * 128:(j + 1) * 128],
                                    xcs[j][:, ct * 128:(ct + 1) * 128], ident_bf)
                            nc.scalar.activation(
                                out=hkvT[ct][:, g * 512:(g + 1) * 512],
                                in_=ps, func=AF.Identity,
                                scale=g1_s[:, ct:ct + 1], bias=b1_s[:, ct:ct + 1])
                        if g < NO // 512:  # own tokens: raw x^T for residual
                            for ct in range(CT):
                                ps32 = sps.tile([128, 512], F32, tag="ps")
                                for j in range(4):
                                    nc.tensor.transpose(
                                        ps32[:, j * 128:(j + 1) * 128],
                                        xts[j][:, ct * 128:(ct + 1) * 128],
                                        ident)
                                nc.vector.tensor_copy(
                                    out=xoT[ct][:, g * 512:(g + 1) * 512],
                                    in_=ps32)

                # ---- V for all kv tiles (vA needed before any AV matmul) --
                for nt in range(KT):
                    psA = mps.tile([128, 512], F32, tag="mp", name="psA")
                    psB = mps.tile([128, 256], F32, tag="mp", name="psB")
                    for ct in range(CT):
                        hk = hkvT[ct][:, nt * 128:(nt + 1) * 128]
                        nc.tensor.matmul(psA, hk, wq[:, ct, 2 * C:2 * C + 512],
                                         start=(ct == 0), stop=(ct == CT - 1))
                        nc.tensor.matmul(psB, hk, wq[:, ct, 2 * C + 512:3 * C],
                                         start=(ct == 0), stop=(ct == CT - 1))
                    nc.vector.tensor_copy(
                        out=vA[nt][:, 0:8, 0:D],
                        in_=psA.rearrange("p (h d) -> p h d", d=D))
                    nc.vector.tensor_copy(
                        out=vA[nt][:, 8:12, 0:D],
                        in_=psB.rearrange("p (h d) -> p h d", d=D))
                    nc.vector.memset(vA[nt][:, :, D:D + 1], 1.0)

                # ---- K and Q for one feature tile ----
                def kq_ft(ft):
                    # mps, not sps: during attention the proj/MLP psum pool
                    # is idle, and using sps here would stall the S->exp
                    # pipeline on slot rotation.
                    for chk in range(NKV // 512):
                        ps = mps.tile([128, 512], F32, tag="mp")
                        for ct in range(CT):
                            nc.tensor.matmul(
                                ps,
                                wq[:, ct, C + ft * 128:C + (ft + 1) * 128],
                                hkvT[ct][:, chk * 512:(chk + 1) * 512],
                                start=(ct == 0), stop=(ct == CT - 1))
                        nc.vector.tensor_copy(
                            out=kT[ft][:, chk * 512:(chk + 1) * 512], in_=ps)
                    for chk in range(QCH):
                        ps = mps.tile([128, 512], F32, tag="mp")
                        for ct in range(CT):
                            nc.tensor.matmul(
                                ps,
                                wq[:, ct, ft * 128:(ft + 1) * 128],
                                hkvT[ct][:, chk * 512:(chk + 1) * 512],
                                start=(ct == 0), stop=(ct == CT - 1))
                        nc.vector.tensor_copy(
                            out=qT[ft][:, chk * 512:(chk + 1) * 512], in_=ps)

                kq_ft(0)
                kq_ft(1)  # lead-in: Scalar is still on LN1, keep PE dense
                # attn(ch0): K/Q of later feature tiles fill the PE while
                # the Scalar engine drains each head pair's exps.
                attn_chunk(0, [lambda f=f: kq_ft(f) for f in range(2, CT)])

            # hkvT/wq freed; normalize ch0, then run attn(ch1) with
            # proj(ch0)/LN2(ch0)/fc1(ch0,hb0) as PE fillers.
            ot_norm(0)

            if True:
                wp_pool = pool(name="wproj", bufs=1, side="right")
                w1_pool = pool(name="wfc1", bufs=1, side="right")
                gbuf = pool(name="gbuf", bufs=13, side="right")
                h2a_pool = pool(name="h2a_sb", bufs=1, side="right")
                acc_pool = pool(name="acc_sb", bufs=1, side="right")
                bc_pool = pool(name="bc_sb", bufs=1, side="right")
                lw = pool(name="mlp_work", bufs=2, side="right")
                wp = wp_pool.tile([128, CT, C], BF16, tag="wproj")
                dma(out=wp, in_=wprojT.rearrange("(ct p) f -> p ct f", p=128))
                HH = HID // 2
                w1t0 = w1_pool.tile([128, CT, HH], BF16, tag="wfc1",
                                    name="w1h0")
                dma(out=w1t0,
                    in_=wfc1T.rearrange("(ct p) f -> p ct f", p=128)[:, :, 0:HH])

                h1s0 = [None] * HQ  # staged pre-gelu fc1 outputs (ch0, hb0)

                # proj/LN2/h2 for chunk 0 between the chunks: the Scalar
                # engine's exp backlog drains here, and the LN2 sqrt's
                # table switch lands outside the exp stream.
                proj_piece(0, range(CT), wp)
                ln2_chunk(0, lw)
                h2_prep(0, lw, h2a_pool, bc_pool)

                fillers1 = [
                    lambda: fc1_piece(0, 0, range(0, 3), w1t0, gbuf, h1s0),
                    lambda: fc1_piece(0, 0, range(3, 6), w1t0, gbuf, h1s0),
                    lambda: fc1_piece(0, 0, range(6, 9), w1t0, gbuf, h1s0),
                    lambda: fc1_piece(0, 0, range(9, HQ), w1t0, gbuf, h1s0),
                    None,
                    None,
                ]
                attn_chunk(1, fillers1)
                _stk.close()  # free qT/kT/vA/pt pools for the MLP tail
                w2_pool = pool(name="wfc2", bufs=1, side="right")
                h2b_pool = pool(name="h2b_sb", bufs=1, side="right")
                accb_pool = pool(name="accb_sb", bufs=1, side="right")
                osb = pool(name="out_sb", bufs=2, side="right")
                w2t0 = w2_pool.tile([128, HQ, C], BF16, tag="wfc2",
                                    name="w2h0")
                dma(out=w2t0,
                    in_=wfc2T.rearrange("(ht p) f -> p ht f", p=128)
                    [:, 0:HQ, :])

                # gelu(ch0,hb0) now that attention exps are done; fc2 covers
                # the PE while ch1's softmax normalization drains.
                g00 = [None] * HQ
                gelu_piece(0, 0, range(HQ), gbuf, h1s0, g00)
                ot_norm(1)
                fc2_piece(0, 0, range(CT), w2t0, g00, acc_pool, None)
                proj_piece(1, range(CT), wp)
                ln2_chunk(1, lw)
                h2_prep(1, lw, h2b_pool, bc_pool)

                # MLP(ch1, hb0)
                g10 = [None] * HQ
                fc1_full(1, 0, w1t0, gbuf, g10)
                w1t1 = w1_pool.tile([128, CT, HH], BF16, tag="wfc1",
                                    name="w1h1")
                dma(out=w1t1,
                    in_=wfc1T.rearrange("(ct p) f -> p ct f", p=128)
                    [:, :, HH:HID])
                fc2_piece(1, 0, range(CT), w2t0, g10, accb_pool, None)
                w2t1 = w2_pool.tile([128, HQ, C], BF16, tag="wfc2",
                                    name="w2h1")
                dma(out=w2t1,
                    in_=wfc2T.rearrange("(ht p) f -> p ht f", p=128)
                    [:, HQ:HT, :])

                # hb1 for both chunks
                g01 = [None] * HQ
                fc1_full(0, 1, w1t1, gbuf, g01)
                fc2_piece(0, 1, range(CT), w2t1, g01, None, osb)
                g11 = [None] * HQ
                fc1_full(1, 1, w1t1, gbuf, g11)
                fc2_piece(1, 1, range(CT), w2t1, g11, None, osb)


def kernel(**inputs):
    global _NC_CACHE, LAST_RESULTS
    import os
    ins = {k: np.ascontiguousarray(np.asarray(v, dtype=np.float32))
           for k, v in inputs.items()}
    if _NC_CACHE is None:
        _NC_CACHE = build_program()
    nc = _NC_CACHE

    import ml_dtypes
    bf = ml_dtypes.bfloat16
    shared = {
        "wqkvT": np.ascontiguousarray(ins["qkv_w"].T.astype(bf)),
        "wprojT": np.ascontiguousarray(ins["proj_w"].T.astype(bf)),
        "wfc1T": np.ascontiguousarray(ins["fc1_w"].T.astype(bf)),
        "wfc2T": np.ascontiguousarray(ins["fc2_w"].T.astype(bf)),
        "pb": ins["proj_b"], "f1b": ins["fc1_b"], "f2b": ins["fc2_b"],
        "g1": ins["ln1_g"], "b1": ins["ln1_b"],
        "g2": ins["ln2_g"], "b2": ins["ln2_b"],
    }
    in_maps = []
    for s in range(NCORES):
        b, half = s // 2, s % 2
        m = dict(shared)
        m["xb"] = np.ascontiguousarray(np.roll(ins["x"][b], -half * NO, axis=0))
        in_maps.append(m)

    trace = bool(int(os.environ.get("KBENCH_TRACE", "0")))
    LAST_RESULTS = run_bass_kernel_spmd(
        nc, in_maps, core_ids=list(range(NCORES)), trace=trace)
    out = np.empty((B, N, C), np.float32)
    for s in range(NCORES):
        b, half = s // 2, s % 2
        out[b, half * NO:(half + 1) * NO, :] = LAST_RESULTS.results[s]["outT"].T
    return out


# revision 29
# speedup vs baseline: 1.0230x; 1.0230x over previous
"""Trainium2 Bass kernel for a dense transformer block (B=4, N=2048, C=768,
H=12, D=64, HID=3072), sharded over 8 NeuronCores.

Sharding: token-split, no collectives. Core s handles batch b = s//2,
sequence half = s%2 (1024 tokens). Each core receives its batch element's
full 2048-token x (rolled so its own tokens are rows 0..1023), computes
K/V over all 2048 tokens (redundantly with its pair core), and produces
the output for its own 1024 tokens. Host gathers/transposes.

v2 schedule: attention is ScalarE(exp)-bound, so the PE instruction
stream is interleaved with independent matmul work ("fillers"): K/Q
feature tiles are computed between chunk-0 head-pairs, and proj/LN2/fc1
(chunk 0) run between chunk-1 head-pairs. S->exp->AV runs software-
pipelined with a 2-step skew so the PE never waits long on the Scalar
engine. Softmax denominators are drained to SBUF ([12,512] per chunk)
and inverted with ONE batched DVE reciprocal per chunk (free-dim-bound
op). All rsqrt/reciprocal-style scalars use Ln+Exp (one activation
table set shared with the softmax exp; Gelu is the only other set).
Residual streams xoT/x2T are bf16 to fit SBUF; PSUM pools are 3+3+2
banks (S tiles / AV accumulators / proj-LN2-MLP).
"""

from contextlib import ExitStack

import numpy as np

import concourse.bass as bass
import concourse.mybir as mybir
import concourse.tile as tile
from concourse import bacc
from concourse.bass_utils import run_bass_kernel_spmd
from concourse.masks import make_identity

F32 = mybir.dt.float32
F32R = mybir.dt.float32r
BF16 = mybir.dt.bfloat16
AF = mybir.ActivationFunctionType
ALU = mybir.AluOpType

B, N, C = 4, 2048, 768
H, D = 12, 64
HID = 3072
EPS = 1e-5
NCORES = 8
NO = 1024  # tokens owned per core
NKV = 2048  # key/value tokens per core
CT = C // 128  # 6 feature tiles
HT = HID // 128  # 24 hidden tiles
HQ = HT // 2  # 12 hidden tiles per half
KT = NKV // 128  # 16 kv token tiles
QCH = NO // 512  # 2 query chunks of 512
ISCALE = 1.0 / np.sqrt(D)
SKEW = 2  # S->AV software-pipeline depth (in nt steps)

LAST_RESULTS = None
_NC_CACHE = None


def build_program(repeats=1):
    nc = bacc.Bacc(trn_type="TRN2", target_bir_lowering=False, num_devices=NCORES)

    xb = nc.dram_tensor("xb", [NKV, C], F32, kind="ExternalInput").ap()
    wqkvT = nc.dram_tensor("wqkvT", [C, 3 * C], BF16, kind="ExternalInput").ap()
    wprojT = nc.dram_tensor("wprojT", [C, C], BF16, kind="ExternalInput").ap()
    wfc1T = nc.dram_tensor("wfc1T", [C, HID], BF16, kind="ExternalInput").ap()
    wfc2T = nc.dram_tensor("wfc2T", [HID, C], BF16, kind="ExternalInput").ap()
    pb = nc.dram_tensor("pb", [C], F32, kind="ExternalInput").ap()
    f1b = nc.dram_tensor("f1b", [HID], F32, kind="ExternalInput").ap()
    f2b = nc.dram_tensor("f2b", [C], F32, kind="ExternalInput").ap()
    g1 = nc.dram_tensor("g1", [C], F32, kind="ExternalInput").ap()
    b1 = nc.dram_tensor("b1", [C], F32, kind="ExternalInput").ap()
    g2 = nc.dram_tensor("g2", [C], F32, kind="ExternalInput").ap()
    b2 = nc.dram_tensor("b2", [C], F32, kind="ExternalInput").ap()
    outT = nc.dram_tensor("outT", [C, NO], F32, kind="ExternalOutput").ap()

    with tile.TileContext(nc) as tc:
        for _ in range(repeats):
            emit(nc, tc, xb, wqkvT, wprojT, wfc1T, wfc2T, pb, f1b, f2b,
                 g1, b1, g2, b2, outT)
    nc.compile()
    return nc


def emit(nc, tc, xb, wqkvT, wprojT, wfc1T, wfc2T, pb, f1b, f2b,
         g1, b1, g2, b2, outT):
    dma = nc.sync.dma_start
    qs = (slice(0, 64), slice(64, 128))
    qs2 = qs

    with ExitStack() as _st:
        def pool(**kw):
            return _st.enter_context(tc.tile_pool(**kw))

        consts = pool(name="consts", bufs=1)
        x2T_pool = pool(name="x2T", bufs=1)
        xoT_pool = pool(name="xoT", bufs=1)
        # ---- constants ----
        ident = consts.tile([128, 128], F32, tag="ident")
        make_identity(nc, ident)
        ident_bf = consts.tile([128, 128], BF16, tag="ident_bf")
        make_identity(nc, ident_bf)
        ones_f32 = consts.tile([128, 1], F32, tag="ones_f32")
        nc.vector.memset(ones_f32, 1.0)
        ones_cb = consts.tile([128, 1], BF16, tag="ones_cb")
        nc.scalar.activation(out=ones_cb, in_=ones_f32, func=AF.Copy)
        eps_t = consts.tile([128, 1], F32, tag="eps")
        nc.vector.memset(eps_t, EPS)
        g1_s = consts.tile([128, CT], F32, tag="g1")
        nc.scalar.dma_start(out=g1_s, in_=g1.rearrange("(ct p) -> p ct", p=128))
        b1_s = consts.tile([128, CT], F32, tag="b1")
        nc.scalar.dma_start(out=b1_s, in_=b1.rearrange("(ct p) -> p ct", p=128))
        g2_s = consts.tile([128, CT], F32, tag="g2")
        nc.scalar.dma_start(out=g2_s, in_=g2.rearrange("(ct p) -> p ct", p=128))
        b2_s = consts.tile([128, CT], F32, tag="b2")
        nc.scalar.dma_start(out=b2_s, in_=b2.rearrange("(ct p) -> p ct", p=128))
        pb_s = consts.tile([128, CT], F32, tag="pb")
        nc.scalar.dma_start(out=pb_s, in_=pb.rearrange("(ct p) -> p ct", p=128))
        f2b_s = consts.tile([128, CT], F32, tag="f2b")
        nc.scalar.dma_start(out=f2b_s, in_=f2b.rearrange("(ct p) -> p ct", p=128))
        f1b_s = consts.tile([128, HT], F32, tag="f1b")
        nc.scalar.dma_start(out=f1b_s, in_=f1b.rearrange("(ht p) -> p ht", p=128))

        # residual streams (bf16 to fit SBUF; rel-err budget is 2e-2)
        x2T = [x2T_pool.tile([128, NO], BF16, tag=f"x2T{ct}", name=f"x2T{ct}")
               for ct in range(CT)]
        xoT = [xoT_pool.tile([128, NO], BF16, tag=f"xoT{ct}", name=f"xoT{ct}")
               for ct in range(CT)]

        if True:
            oU_pool = pool(name="oU", bufs=12)
            oT_pool = pool(name="oT", bufs=6)
            den_pool = pool(name="den", bufs=1)
            asm = pool(name="asm", bufs=2)
            sps = pool(name="s_psum", bufs=2, space="PSUM")
            ops = pool(name="o_psum", bufs=2, space="PSUM")
            mps = pool(name="m_psum", bufs=2, space="PSUM")

            _stk = ExitStack()  # last-opened left pools: freed after attention

            def kpool(**kw):
                return _stk.enter_context(tc.tile_pool(**kw))

            qT_pool = kpool(name="qT", bufs=1)
            kT_pool = kpool(name="kT", bufs=1)
            vA_pool = kpool(name="vA", bufs=1)
            p_sb = kpool(name="p_sb", bufs=4)
            qT = [qT_pool.tile([128, NO], BF16, tag=f"qT{ct}", name=f"qT{ct}")
                  for ct in range(CT)]
            kT = [kT_pool.tile([128, NKV], BF16, tag=f"kT{ct}", name=f"kT{ct}")
                  for ct in range(CT)]
            vA = [vA_pool.tile([128, H, D + 1], BF16, tag=f"vA{nt}",
                               name=f"vA{nt}") for nt in range(KT)]
            dent = [None, None]
            recT = [None, None]
            # oT[ch][hp]: normalized attention output, feature-major
            oT = [[None] * CT for _ in range(QCH)]
            ln2v = [None, None]

            # ---------------- attention inner machinery ----------------
            def attn_hp(ch, hp):
                """S->exp->AV pipeline for head pair hp, query chunk ch."""
                qch = slice(ch * 512, (ch + 1) * 512)
                po = [ops.tile([D + 1, 512], F32, tag="po",
                               name=f"po{ch}_{hp}_{i}") for i in range(2)]
                pts = {}
                for step in range(KT + SKEW):
                    if step < KT:
                        nt = step
                        ps2 = sps.tile([128, 1024], F32, tag="ps")
                        for i in range(2):
                            nc.tensor.matmul(
                                ps2[:, i * 512:(i + 1) * 512],
                                kT[hp][qs[i], nt * 128:(nt + 1) * 128],
                                qT[hp][qs[i], qch],
                                start=True, stop=True,
                                tile_position=(64 * i, 0))
                        pt2 = p_sb.tile([128, 1024], BF16, tag="pt")
                        nc.scalar.activation(out=pt2, in_=ps2,
                                             func=AF.Exp, scale=ISCALE)
                        pts[nt] = pt2
                    if step >= SKEW:
                        m = step - SKEW
                        pt2 = pts.pop(m)
                        for i in range(2):
                            nc.tensor.matmul(
                                po[i], vA[m][:, 2 * hp + i, :],
                                pt2[:, i * 512:(i + 1) * 512],
                                start=(m == 0), stop=(m == KT - 1),
                                skip_group_check=True)
                # drain: unnormalized O^T halves + denominator row
                oUs = []
                for i in range(2):
                    r = 2 * hp + i
                    oU = oU_pool.tile([D, 512], BF16, tag="oU",
                                      name=f"oU{ch}_{r}")
                    nc.vector.tensor_copy(out=oU, in_=po[i][0:D, :])
                    oUs.append(oU)
                    dt = asm.tile([1, 512], BF16, tag="dtmp")
                    nc.vector.tensor_copy(out=dt, in_=po[i][D:D + 1, :])
                    dma(out=dent[ch][r:r + 1, :], in_=dt)
                return oUs

            oU_all = [[None] * H for _ in range(QCH)]

            def attn_chunk(ch, fillers):
                dent[ch] = den_pool.tile([H, 512], BF16, tag="dent",
                                         name=f"dent{ch}")
                for hp in range(CT):
                    oUs = attn_hp(ch, hp)
                    oU_all[ch][2 * hp] = oUs[0]
                    oU_all[ch][2 * hp + 1] = oUs[1]
                    if hp < len(fillers) and fillers[hp] is not None:
                        fillers[hp]()

            def ot_norm(ch):
                """One batched reciprocal, then per-head broadcast+mul."""
                recT[ch] = den_pool.tile([H, 512], BF16, tag="recT",
                                         name=f"recT{ch}")
                with nc.allow_low_precision(reason="softmax denom in bf16"):
                    nc.vector.reciprocal(out=recT[ch], in_=dent[ch])
                for hp in range(CT):
                    t = oT_pool.tile([128, 512], BF16, tag="oT",
                                     name=f"oT{ch}_{hp}")
                    for i in range(2):
                        r = 2 * hp + i
                        rb = asm.tile([1, 512], BF16, tag="rb")
                        dma(out=rb, in_=recT[ch][r:r + 1, :])
                        vb = asm.tile([D, 512], BF16, tag="vb")
                        nc.gpsimd.partition_broadcast(vb, rb, channels=D)
                        nc.vector.tensor_mul(t[qs2[i], :], oU_all[ch][r], vb)
                    oT[ch][hp] = t

            # ------------- proj / LN2 / MLP building blocks -------------
            def proj_piece(ch, fts, wp):
                cs = slice(ch * 512, (ch + 1) * 512)
                for ft in fts:
                    ps = mps.tile([128, 512], F32, tag="mp", name="ppj")
                    for ct in range(CT):
                        nc.tensor.matmul(
                            ps, wp[:, ct, ft * 128:(ft + 1) * 128],
                            oT[ch][ct],
                            start=(ct == 0), stop=(ct == CT - 1))
                    nc.vector.scalar_tensor_tensor(
                        out=x2T[ft][:, cs],
                        in0=ps, scalar=pb_s[:, ft:ft + 1],
                        in1=xoT[ft][:, cs],
                        op0=ALU.add, op1=ALU.add)

            def ln2_chunk(ch, lw):
                cs = slice(ch * 512, (ch + 1) * 512)
                psum = mps.tile([1, 512], F32, tag="mp", name="psum")
                pssq = mps.tile([1, 512], F32, tag="mp", name="pssq")
                for ct in range(CT):
                    sq = lw.tile([128, 512], BF16, tag="sq", bufs=1)
                    nc.vector.tensor_mul(sq, x2T[ct][:, cs], x2T[ct][:, cs])
                    nc.tensor.matmul(psum, ones_cb, x2T[ct][:, cs],
                                     start=(ct == 0), stop=(ct == CT - 1),
                                     skip_group_check=True)
                    nc.tensor.matmul(pssq, ones_cb, sq,
                                     start=(ct == 0), stop=(ct == CT - 1),
                                     skip_group_check=True)
                mu = lw.tile([1, 512], F32, tag="mu", name=f"mu{ch}", bufs=1)
                nc.scalar.mul(mu, psum, 1.0 / C)
                msq = lw.tile([1, 512], F32, tag="msq", bufs=1)
                nc.scalar.mul(msq, pssq, 1.0 / C)
                mu2 = lw.tile([1, 512], F32, tag="mu2", bufs=1)
                nc.vector.tensor_mul(mu2, mu, mu)
                nc.vector.tensor_sub(msq, msq, mu2)  # msq := var
                nc.scalar.activation(out=mu2, in_=msq, func=AF.Sqrt,
                                     bias=eps_t[0:1], scale=1.0)
                rstdf = lw.tile([1, 512], F32, tag="rstdf", bufs=1)
                nc.vector.reciprocal(out=rstdf, in_=mu2)
                rstd = lw.tile([1, 512], BF16, tag=f"rstd2_{ch}",
                               name="rstd2", bufs=1)
                nc.vector.tensor_copy(out=rstd, in_=rstdf)
                nmr = lw.tile([1, 512], BF16, tag=f"nmr2_{ch}",
                              name="nmr2", bufs=1)
                nc.vector.scalar_tensor_tensor(
                    out=nmr, in0=mu, scalar=-1.0, in1=rstdf,
                    op0=ALU.mult, op1=ALU.mult)
                ln2v[ch] = (rstd, nmr)

            h2c = [[None] * CT for _ in range(QCH)]

            def h2_prep(ch, lw, h2_pool, bc_pool):
                cs = slice(ch * 512, (ch + 1) * 512)
                rstd, nmr = ln2v[ch]
                bc_r = bc_pool.tile([128, 512], BF16, tag="bc_r")
                nc.gpsimd.partition_broadcast(bc_r, rstd, channels=128)
                bc_m = bc_pool.tile([128, 512], BF16, tag="bc_m")
                nc.gpsimd.partition_broadcast(bc_m, nmr, channels=128)
                for ct in range(CT):
                    t = lw.tile([128, 512], BF16, tag="h2tmp")
                    nc.vector.tensor_mul(t, x2T[ct][:, cs], bc_r)
                    nc.vector.tensor_add(t, t, bc_m)
                    h2 = h2_pool.tile([128, 512], BF16,
                                      tag=f"h2_{ct}_{ch}", name=f"h2_{ct}_{ch}")
                    nc.scalar.activation(
                        out=h2, in_=t, func=AF.Identity,
                        scale=g2_s[:, ct:ct + 1], bias=b2_s[:, ct:ct + 1])
                    h2c[ch][ct] = h2

            # fc1 matmuls only (stage pre-gelu h1 in bf16; gelu deferred so
            # the Scalar engine stays on the exp table set during attention)
            def fc1_piece(ch, hb, hts, w1t, gbuf, h1s):
                for ht in hts:
                    ps = mps.tile([128, 512], F32, tag="mp", name="f1")
                    for ct in range(CT):
                        nc.tensor.matmul(
                            ps, w1t[:, ct, ht * 128:(ht + 1) * 128],
                            h2c[ch][ct],
                            start=(ct == 0), stop=(ct == CT - 1))
                    h1 = gbuf.tile([128, 512], BF16, tag="gb",
                                   name=f"h1_{ch}_{hb}_{ht}")
                    nc.vector.tensor_copy(out=h1, in_=ps)
                    h1s[ht] = h1

            def gelu_piece(ch, hb, hts, gbuf, h1s, gs):
                for ht in hts:
                    g = gbuf.tile([128, 512], BF16, tag="gb",
                                  name=f"g_{ch}_{hb}_{ht}")
                    hti = hb * HQ + ht
                    nc.scalar.activation(out=g, in_=h1s[ht], func=AF.Gelu,
                                         bias=f1b_s[:, hti:hti + 1], scale=1.0)
                    gs[ht] = g

            # fc1 with gelu fused right after (for post-attention phases)
            def fc1_full(ch, hb, w1t, gbuf, gs):
                for ht in range(HQ):
                    ps = mps.tile([128, 512], F32, tag="mp", name="f1")
                    for ct in range(CT):
                        nc.tensor.matmul(
                            ps, w1t[:, ct, ht * 128:(ht + 1) * 128],
                            h2c[ch][ct],
                            start=(ct == 0), stop=(ct == CT - 1))
                    g = gbuf.tile([128, 512], BF16, tag="gb",
                                  name=f"g_{ch}_{hb}_{ht}")
                    hti = hb * HQ + ht
                    nc.scalar.activation(out=g, in_=ps, func=AF.Gelu,
                                         bias=f1b_s[:, hti:hti + 1], scale=1.0)
                    gs[ht] = g

            acc = {}

            def fc2_piece(ch, hb, fts, w2t, gs, accp, osb):
                cs = slice(ch * 512, (ch + 1) * 512)
                for ft in fts:
                    ps = mps.tile([128, 512], F32, tag="mp", name="f2")
                    for ht in range(HQ):
                        nc.tensor.matmul(
                            ps, w2t[:, ht, ft * 128:(ft + 1) * 128], gs[ht],
                            start=(ht == 0), stop=(ht == HQ - 1))
                    if hb == 0:
                        a = accp.tile([128, 512], BF16, tag=f"acc{ft}_{ch}",
                                      name=f"acc{ft}_{ch}")
                        acc[(ft, ch)] = a
                        nc.vector.scalar_tensor_tensor(
                            out=a, in0=ps,
                            scalar=f2b_s[:, ft:ft + 1],
                            in1=x2T[ft][:, cs],
                            op0=ALU.add, op1=ALU.add)
                    else:
                        ot = osb.tile([128, 512], F32, tag="ot")
                        nc.vector.tensor_add(ot, ps, acc[(ft, ch)])
                        dma(out=outT[ft * 128:(ft + 1) * 128, cs], in_=ot)

            # ============ Phase 1+2: LN1, transposes, QKV, attn(ch0) ======
            with (
                tc.tile_pool(name="hkvT", bufs=1) as hkvT_pool,
                tc.tile_pool(name="wqkv", bufs=1) as wq_pool,
            ):
                hkvT = [hkvT_pool.tile([128, NKV], BF16, tag=f"hkvT{ct}",
                                       name=f"hkvT{ct}") for ct in range(CT)]
                wq = wq_pool.tile([128, CT, 3 * C], BF16, tag="wqkv")
                with (
                    tc.tile_pool(name="ln1_work", bufs=2) as lw1,
                    tc.tile_pool(name="ln1_stat", bufs=6) as lstat,
                ):
                    for g in range(KT // 4):  # groups of 4 token tiles
                        xts, xcs = [], []
                        for j in range(4):
                            nt = 4 * g + j
                            xt = lw1.tile([128, C], F32, tag=f"xt{j}",
                                          name=f"xt{j}")
                            dma(out=xt, in_=xb[nt * 128:(nt + 1) * 128, :])
                            st = lstat.tile([128, 3, 6], F32, tag="st")
                            xg = xt.rearrange("p (s d) -> p s d", s=3)
                            for s in range(3):
                                nc.vector.bn_stats(out=st[:, s], in_=xg[:, s])
                            mv = lstat.tile([128, 2], F32, tag="mv")
                            nc.vector.bn_aggr(out=mv, in_=st)
                            rstd = lstat.tile([128, 1], F32, tag="rstd")
                            nc.scalar.activation(out=rstd, in_=mv[:, 1:2],
                                                 func=AF.Sqrt,
                                                 bias=eps_t, scale=1.0)
                            nc.vector.reciprocal(out=rstd, in_=rstd)
                            nmr = lstat.tile([128, 1], F32, tag="nmr")
                            nc.vector.tensor_scalar(out=nmr, in0=mv[:, 0:1],
                                                    scalar1=-1.0, scalar2=rstd,
                                                    op0=ALU.mult, op1=ALU.mult)
                            xc = lw1.tile([128, C], BF16, tag=f"xc{j}",
                                          name=f"xc{j}", bufs=1)
                            nc.scalar.activation(out=xc, in_=xt,
                                                 func=AF.Identity,
                                                 scale=rstd, bias=nmr)
                            xts.append(xt)
                            xcs.append(xc)
                        if g == 0:
                            dma(out=wq,
                                in_=wqkvT.rearrange("(ct p) f -> p ct f", p=128))
                        for ct in range(CT):
                            ps = sps.tile([128, 512], BF16, tag="ps")
                            for j in range(4):
                                nc.tensor.transpose(
                                    ps[:, j * 128:(j + 1) * 128],
                                    xcs[j][:, ct * 128:(ct + 1) * 128], ident_bf)
                            nc.scalar.activation(
                                out=hkvT[ct][:, g * 512:(g + 1) * 512],
                                in_=ps, func=AF.Identity,
                                scale=g1_s[:, ct:ct + 1], bias=b1_s[:, ct:ct + 1])
                        if g < NO // 512:  # own tokens: raw x^T for residual
                            for ct in range(CT):
                                ps32 = sps.tile([128, 512], F32, tag="ps")
                                for j in range(4):
                                    nc.tensor.transpose(
                                        ps32[:, j * 128:(j + 1) * 128],
                                        xts[j][:, ct * 128:(ct + 1) * 128],
                                        ident)
                                nc.vector.tensor_copy(
                                    out=xoT[ct][:, g * 512:(g + 1) * 512],
                                    in_=ps32)

                # ---- V for all kv tiles (vA needed before any AV matmul) --
                for nt in range(KT):
                    psA = mps.tile([128, 512], F32, tag="mp", name="psA")
                    psB = mps.tile([128, 256], F32, tag="mp", name="psB")
                    for ct in range(CT):
                        hk = hkvT[ct][:, nt * 128:(nt + 1) * 128]
                        nc.tensor.matmul(psA, hk, wq[:, ct, 2 * C:2 * C + 512],
                                         start=(ct == 0), stop=(ct == CT - 1))
                        nc.tensor.matmul(psB, hk, wq[:, ct, 2 * C + 512:3 * C],
                                         start=(ct == 0), stop=(ct == CT - 1))
                    nc.vector.tensor_copy(
                        out=vA[nt][:, 0:8, 0:D],
                        in_=psA.rearrange("p (h d) -> p h d", d=D))
                    nc.vector.tensor_copy(
                        out=vA[nt][:, 8:12, 0:D],
                        in_=psB.rearrange("p (h d) -> p h d", d=D))
                    nc.vector.memset(vA[nt][:, :, D:D + 1], 1.0)

                # ---- K and Q for one feature tile ----
                def kq_ft(ft):
                    # mps, not sps: during attention the proj/MLP psum pool
                    # is idle, and using sps here would stall the S->exp
                    # pipeline on slot rotation.
                    for chk in range(NKV // 512):
                        ps = mps.tile([128, 512], F32, tag="mp")
                        for ct in range(CT):
                            nc.tensor.matmul(
                                ps,
                                wq[:, ct, C + ft * 128:C + (ft + 1) * 128],
                                hkvT[ct][:, chk * 512:(chk + 1) * 512],
                                start=(ct == 0), stop=(ct == CT - 1))
                        nc.vector.tensor_copy(
                            out=kT[ft][:, chk * 512:(chk + 1) * 512], in_=ps)
                    for chk in range(QCH):
                        ps = mps.tile([128, 512], F32, tag="mp")
                        for ct in range(CT):
                            nc.tensor.matmul(
                                ps,
                                wq[:, ct, ft * 128:(ft + 1) * 128],
                                hkvT[ct][:, chk * 512:(chk + 1) * 512],
                                start=(ct == 0), stop=(ct == CT - 1))
                        nc.vector.tensor_copy(
                            out=qT[ft][:, chk * 512:(chk + 1) * 512], in_=ps)

                kq_ft(0)
                kq_ft(1)  # lead-in: Scalar is still on LN1, keep PE dense
                # attn(ch0): K/Q of later feature tiles fill the PE while
                # the Scalar engine drains each head pair's exps.
                attn_chunk(0, [lambda f=f: kq_ft(f) for f in range(2, CT)])

            # hkvT/wq freed; normalize ch0, then run attn(ch1) with
            # proj(ch0)/LN2(ch0)/fc1(ch0,hb0) as PE fillers.
            ot_norm(0)

            if True:
                wp_pool = pool(name="wproj", bufs=1, side="right")
                w1_pool = pool(name="wfc1", bufs=1, side="right")
                gbuf = pool(name="gbuf", bufs=13, side="right")
                h2a_pool = pool(name="h2a_sb", bufs=1, side="right")
                acc_pool = pool(name="acc_sb", bufs=1, side="right")
                bc_pool = pool(name="bc_sb", bufs=1, side="right")
                lw = pool(name="mlp_work", bufs=2, side="right")
                wp = wp_pool.tile([128, CT, C], BF16, tag="wproj")
                dma(out=wp, in_=wprojT.rearrange("(ct p) f -> p ct f", p=128))
                HH = HID // 2
                w1t0 = w1_pool.tile([128, CT, HH], BF16, tag="wfc1",
                                    name="w1h0")
                dma(out=w1t0,
                    in_=wfc1T.rearrange("(ct p) f -> p ct f", p=128)[:, :, 0:HH])

                h1s0 = [None] * HQ  # staged pre-gelu fc1 outputs (ch0, hb0)

                # proj/LN2/h2 for chunk 0 between the chunks: the Scalar
                # engine's exp backlog drains here, and the LN2 sqrt's
                # table switch lands outside the exp stream.
                proj_piece(0, range(CT), wp)
                ln2_chunk(0, lw)
                h2_prep(0, lw, h2a_pool, bc_pool)

                fillers1 = [
                    lambda: fc1_piece(0, 0, range(0, 3), w1t0, gbuf, h1s0),
                    lambda: fc1_piece(0, 0, range(3, 6), w1t0, gbuf, h1s0),
                    lambda: fc1_piece(0, 0, range(6, 9), w1t0, gbuf, h1s0),
                    lambda: fc1_piece(0, 0, range(9, HQ), w1t0, gbuf, h1s0),
                    None,
                    None,
                ]
                attn_chunk(1, fillers1)
                _stk.close()  # free qT/kT/vA/pt pools for the MLP tail
                w2_pool = pool(name="wfc2", bufs=1, side="right")
                h2b_pool = pool(name="h2b_sb", bufs=1, side="right")
                accb_pool = pool(name="accb_sb", bufs=1, side="right")
                osb = pool(name="out_sb", bufs=2, side="right")
                w2t0 = w2_pool.tile([128, HQ, C], BF16, tag="wfc2",
                                    name="w2h0")
                dma(out=w2t0,
                    in_=wfc2T.rearrange("(ht p) f -> p ht f", p=128)
                    [:, 0:HQ, :])

                # gelu(ch0,hb0) now that attention exps are done; fc2 covers
                # the PE while ch1's softmax normalization drains.
                g00 = [None] * HQ
                gelu_piece(0, 0, range(HQ), gbuf, h1s0, g00)
                ot_norm(1)
                fc2_piece(0, 0, range(CT), w2t0, g00, acc_pool, None)
                proj_piece(1, range(CT), wp)
                ln2_chunk(1, lw)
                h2_prep(1, lw, h2b_pool, bc_pool)

                # MLP(ch1, hb0)
                g10 = [None] * HQ
                fc1_full(1, 0, w1t0, gbuf, g10)
                w1t1 = w1_pool.tile([128, CT, HH], BF16, tag="wfc1",
                                    name="w1h1")
                dma(out=w1t1,
                    in_=wfc1T.rearrange("(ct p) f -> p ct f", p=128)
                    [:, :, HH:HID])
                fc2_piece(1, 0, range(CT), w2t0, g10, accb_pool, None)
                w2t1 = w2_pool.tile([128, HQ, C], BF16, tag="wfc2",
                                    name="w2h1")
                dma(out=w2t1,
                    in_=wfc2T.rearrange("(ht p) f -> p ht f", p=128)
                    [:, HQ:HT, :])

                # hb1 for both chunks
                g01 = [None] * HQ
                fc1_full(0, 1, w1t1, gbuf, g01)
                fc2_piece(0, 1, range(CT), w2t1, g01, None, osb)
                g11 = [None] * HQ
                fc1_full(1, 1, w1t1, gbuf, g11)
                fc2_piece(1, 1, range(CT), w2t1, g11, None, osb)


def kernel(**inputs):
    global _NC_CACHE, LAST_RESULTS
    import os
    ins = {k: np.ascontiguousarray(np.asarray(v, dtype=np.float32))
           for k, v in inputs.items()}
    if _NC_CACHE is None:
        _NC_CACHE = build_program()
    nc = _NC_CACHE

    import ml_dtypes
    bf = ml_dtypes.bfloat16
    shared = {
        "wqkvT": np.ascontiguousarray(ins["qkv_w"].T.astype(bf)),
        "wprojT": np.ascontiguousarray(ins["proj_w"].T.astype(bf)),
        "wfc1T": np.ascontiguousarray(ins["fc1_w"].T.astype(bf)),
        "wfc2T": np.ascontiguousarray(ins["fc2_w"].T.astype(bf)),
        "pb": ins["proj_b"], "f1b": ins["fc1_b"], "f2b": ins["fc2_b"],
        "g1": ins["ln1_g"], "b1": ins["ln1_b"],
        "g2": ins["ln2_g"], "b2": ins["ln2_b"],
    }
    in_maps = []
    for s in range(NCORES):
        b, half = s // 2, s % 2
        m = dict(shared)
        m["xb"] = np.ascontiguousarray(np.roll(ins["x"][b], -half * NO, axis=0))
        in_maps.append(m)

    trace = bool(int(os.environ.get("KBENCH_TRACE", "0")))
    LAST_RESULTS = run_bass_kernel_spmd(
        nc, in_maps, core_ids=list(range(NCORES)), trace=trace)
    out = np.empty((B, N, C), np.float32)
    for s in range(NCORES):
        b, half = s // 2, s % 2
        out[b, half * NO:(half + 1) * NO, :] = LAST_RESULTS.results[s]["outT"].T
    return out
